# revision 11
# baseline (speedup 1.0000x reference)
"""Trainium2 Bass kernel for nn_Model_39676907882504.

Math: qk = (q @ k^T)/8 has shape [1,2048,1,1]; after the transposes it is
[2048,1,1,1], and softmax over the trailing size-1 axis is exactly 1.0
regardless of qk.  The final matmul with attn_weight == 1 reduces to
broadcasting `value` across a new leading dim:

    output[i, j, 0, :] = value[0, j, 0, :]   for all i in [0, 2048)

i.e. a 512KB -> 1GiB broadcast copy.  Pure memory-regime kernel.
Sharding: 256 output rows per core x 8 cores; value staged in SBUF.

HW model (established by trace analysis + probe kernels this session):
  - A dynamic DMA instruction with a 3-dim DRAM-side AP is split into
    PACKETS, one per outermost-dim index; packet j goes to SDMA engine
    64+(j%16), restarting at 64 every instruction.
  - SBUF AXI port p serves partitions ≡ p (mod 16).  If a packet's
    descriptors walk consecutive partitions, engines drift over ports
    and randomly collide (~58% of port rate, measured).  If every
    descriptor of packet j reads THE SAME partition j, engine j is
    locked to port j forever: no collisions ever.
  - Packets of 1 descriptor serialize at ~5.4us (completion latency
    exposed); 7-8 descs per packet stream at port rate.
  - Descriptors cost ~110ns fixed, so keep them 10s-of-KB.
  - Engine 79 (index 15) sustains only ~21.4 GB/s vs ~26.9 for the rest.

Kernel: an output row is cut into P overlapping windows (window p =
vflat[sigma*p : sigma*p + C]; the few overlapping floats rewrite
identical bytes - harmless).  SBUF tile: partition p = window p, single
copy.  One store instruction covers 8 rows as P packets x 8 descriptors:

    out AP [[sigma, P], [ROW_FL, 8], [1, C]]   (packet j = window j of
    in  AP [[C, P], [0, 8], [1, C]]             8 consecutive rows,
                                                re-reading partition j)

  alpha: P=16, sigma=8190, C=8222 - all 16 engines, uniform.
  gamma: P=15, sigma=8738, C=8740 - engines 64-78, engine 79 idle.

26 alpha + 6 gamma instructions cover 256 rows and load engine 79 at
~80% of uniform, matching its speed deficit: all engines finish together
(~320us of stores vs 434us baseline).  Rows split over both HWDGE
queues; each queue ends with an alpha whose semaphore inc is the
queue-drain barrier (its packet on engine j is FIFO-behind everything
earlier on engine j).
"""

import sys

for _p in ("/opt/trn_rl_repo",):
    if _p not in sys.path:
        sys.path.insert(0, _p)

import numpy as np

import bass_rust
import concourse.bass as bass
import concourse.mybir as mybir
from concourse.bass_utils import run_bass_kernel_spmd

S = 2048
D = 64
N_CORES = 8
ROWS_PER_CORE = S // N_CORES          # 256
ROW_FL = S * D                        # 131072 floats per output row
RG = 8                                # rows per instruction (descs/packet)

PA, SGA, CA = 16, 8190, 8222          # alpha: (PA-1)*SGA + CA == ROW_FL
assert (PA - 1) * SGA + CA == ROW_FL
PG, SGG, CG = 15, 8738, 8740          # gamma: engine 79 idle
assert (PG - 1) * SGG + CG == ROW_FL

# per queue: 13 alpha + 3 gamma instructions (16 x 8 rows = 128 rows),
# gamma at these instruction slots; last slot must be alpha (barrier).
GAMMA_SLOTS = {4, 9, 14}

TRACE = False          # test.py flips this to profile
TRACE_KWARGS = {}
LAST_RESULT = None     # BassKernelResults of the last run (for test.py)


def build_program():
    nc = bass.Bass()
    va = nc.declare_dram_parameter("value_a", [PA, CA], mybir.dt.float32,
                                   isOutput=False)
    vg = nc.declare_dram_parameter("value_g", [PG, CG], mybir.dt.float32,
                                   isOutput=False)
    out = nc.declare_dram_parameter("out", [ROWS_PER_CORE, ROW_FL],
                                    mybir.dt.float32, isOutput=True)
    wta = nc.alloc_sbuf_tensor("wta", [PA, CA], mybir.dt.float32)
    wtg = nc.alloc_sbuf_tensor("wtg", [PG, CG], mybir.dt.float32)

    def store(eng, r0, slot):
        o = out[r0:r0 + RG, 0:ROW_FL]
        if slot in GAMMA_SLOTS:
            o.ap = bass_rust.VecI64Pair([[SGG, PG], [ROW_FL, RG], [1, CG]])
            i = wtg[0:PG, 0:CG]
            i.ap = bass_rust.VecI64Pair([[CG, PG], [0, RG], [1, CG]])
        else:
            o.ap = bass_rust.VecI64Pair([[SGA, PA], [ROW_FL, RG], [1, CA]])
            i = wta[0:PA, 0:CA]
            i.ap = bass_rust.VecI64Pair([[CA, PA], [0, RG], [1, CA]])
        return eng.dma_start(out=o, in_=i)

    half = ROWS_PER_CORE // 2
    n_slots = half // RG                              # 16 per queue

    with nc.Block() as block, nc.semaphore("dma_sem") as dma_sem, \
            nc.semaphore("scr_sem") as scr_sem:

        @block.sync
        def _(sync):
            sync.dma_start(out=wta[:, :], in_=va[:, :]).then_inc(dma_sem, 16)
            sync.wait_ge(dma_sem, 32)
            for k in range(n_slots):
                ins = store(sync, k * RG, k)
                ins.then_inc(dma_sem if k == n_slots - 1 else scr_sem, 16)
            sync.wait_ge(dma_sem, 64)

        @block.scalar
        def _(scalar):
            scalar.dma_start(out=wtg[:, :], in_=vg[:, :]).then_inc(dma_sem, 16)
            scalar.wait_ge(dma_sem, 32)
            for k in range(n_slots):
                ins = store(scalar, half + k * RG, k)
                ins.then_inc(dma_sem if k == n_slots - 1 else scr_sem, 16)
            scalar.wait_ge(dma_sem, 64)

    return nc


def _windows(vflat, n, stride, c):
    w = np.zeros((n, c), np.float32)
    for p in range(n):
        w[p] = vflat[p * stride: p * stride + c]
    return w


def kernel(query=None, key=None, value=None, attn_mask=None, **_ignored):
    global LAST_RESULT
    vflat = np.ascontiguousarray(np.asarray(value, np.float32)).reshape(ROW_FL)
    va = _windows(vflat, PA, SGA, CA)
    vg = _windows(vflat, PG, SGG, CG)

    nc = build_program()
    core_ids = list(range(N_CORES))
    in_maps = [{"value_a": va, "value_g": vg} for _ in core_ids]
    res = run_bass_kernel_spmd(nc, in_maps, core_ids, trace=TRACE,
                               **TRACE_KWARGS)
    LAST_RESULT = res

    shards = [res.results[i]["out"].reshape(ROWS_PER_CORE, S, 1, D)
              for i in range(N_CORES)]
    return np.concatenate(shards, axis=0)


# revision 13
# speedup vs baseline: 1.2278x; 1.2278x over previous
"""Trainium2 Bass kernel for nn_Model_39676907882504.

Math: qk = (q @ k^T)/8 has shape [1,2048,1,1]; after the transposes it is
[2048,1,1,1], and softmax over the trailing size-1 axis is exactly 1.0
regardless of qk.  The final matmul with attn_weight == 1 reduces to
broadcasting `value` across a new leading dim:

    output[i, j, 0, :] = value[0, j, 0, :]   for all i in [0, 2048)

i.e. a 512KB -> 1GiB broadcast copy.  Pure memory-regime kernel.
Sharding: 256 output rows per core x 8 cores; value staged in SBUF.

HW model (established by trace analysis + probe kernels this session):
  - A dynamic DMA instruction with a 3-dim DRAM-side AP is split into
    PACKETS, one per outermost-dim index; packet j goes to SDMA engine
    64+(j%16), restarting at 64 every instruction.
  - SBUF AXI port p serves partitions ≡ p (mod 16).  If a packet's
    descriptors walk consecutive partitions, engines drift over ports and
    randomly collide (~58% of port rate, measured).  Packet j reading
    ONLY partition j locks engine j to port j: no collisions ever.
  - Descriptors with IDENTICAL SBUF source addresses serialize at
    ~5.1us each (no pipelining; measured).  Distinct addresses on the
    same partition are required, so each window is replicated RG times
    along the free dim and descriptor i of a packet reads copy i.
  - Packets of 1 descriptor serialize (~5.4us); >=5-8 descs/packet with
    10s-of-KB descriptors stream at port rate (~26.9 GB/s/engine).
  - Engine 79 (index 15) sustains only ~21.4 GB/s vs ~26.9 for the rest.

Kernel: an output row is cut into P overlapping windows (window p =
vflat[sigma*p : sigma*p+C]; overlaps rewrite identical bytes, harmless).
One instruction stores RG rows as P packets x RG descriptors:

    out AP [[sigma, P], [ROW_FL, RG], [1, C]]
    in  AP [[FREE, P], [C, RG], [1, C]]    (partition j, copy i)

  alpha: P=16, sigma=8190, C=8222, RG=6 rows - all 16 engines.
  gamma: P=15, sigma=8738, C=8740, RG=5 rows - engine 79 idle.

Mix 31 alpha + 14 gamma = 256 rows: engine 79 carries 31*6 descs
(286us at 21.4 GB/s) vs 318us on engines 64-78 - balanced within its
speed deficit.  SBUF: one [32, 49332] f32 tile (197 KB/partition):
partitions 0-15 = 6 copies of alpha window p, 16-30 = 5 copies of gamma
window p-16 (port p ≡ partition p mod 16 still).  The load streams as
16 packets x 8 descs of 49332/4 floats.  Each queue ends with an alpha
whose sem inc is the drain barrier (FIFO per engine).
"""

import sys

for _p in ("/opt/trn_rl_repo",):
    if _p not in sys.path:
        sys.path.insert(0, _p)

import numpy as np

import bass_rust
import concourse.bass as bass
import concourse.mybir as mybir
from concourse.bass_utils import run_bass_kernel_spmd

S = 2048
D = 64
N_CORES = 8
ROWS_PER_CORE = S // N_CORES          # 256
ROW_FL = S * D                        # 131072 floats per output row

PA, SGA, CA, RGA = 16, 8190, 8222, 6  # alpha shape
assert (PA - 1) * SGA + CA == ROW_FL
PG, SGG, CG, RGG = 15, 8738, 8740, 5  # gamma shape (engine 79 idle)
assert (PG - 1) * SGG + CG == ROW_FL

FREE = RGA * CA                       # 49332 floats per SBUF partition
assert RGG * CG <= FREE
NPART = 32                            # SBUF partitions (31 used + pad)
LQ = FREE // 4                        # load descriptor: 12333 floats
assert LQ * 4 == FREE and LQ <= 65536

TRACE = False          # test.py flips this to profile
TRACE_KWARGS = {}
LAST_RESULT = None     # BassKernelResults of the last run (for test.py)


def build_program():
    nc = bass.Bass()
    val = nc.declare_dram_parameter("value_w", [NPART * FREE],
                                    mybir.dt.float32, isOutput=False)
    out = nc.declare_dram_parameter("out", [ROWS_PER_CORE, ROW_FL],
                                    mybir.dt.float32, isOutput=True)
    wt = nc.alloc_sbuf_tensor("wt", [NPART, FREE], mybir.dt.float32)

    def store(eng, r0, is_gamma):
        o = out[r0:r0 + (RGG if is_gamma else RGA), 0:ROW_FL]
        if is_gamma:
            o.ap = bass_rust.VecI64Pair([[SGG, PG], [ROW_FL, RGG], [1, CG]])
            i = wt[16:16 + PG, 0:RGG * CG]
            i.ap = bass_rust.VecI64Pair([[FREE, PG], [CG, RGG], [1, CG]])
        else:
            o.ap = bass_rust.VecI64Pair([[SGA, PA], [ROW_FL, RGA], [1, CA]])
            i = wt[0:PA, 0:RGA * CA]
            i.ap = bass_rust.VecI64Pair([[FREE, PA], [CA, RGA], [1, CA]])
        return eng.dma_start(out=o, in_=i)

    def load(eng):
        # 16 packets x 8 descs of LQ floats; SBUF quarter (k%4) of
        # partition (k//4) <- DRAM chunk k (host packs in this order).
        o = wt[0:NPART, 0:FREE]
        o.ap = bass_rust.VecI64Pair([[FREE, NPART], [LQ, 4], [1, LQ]])
        i = val[0:NPART * FREE]
        i.ap = bass_rust.VecI64Pair([[8 * LQ, 16], [LQ, 8], [1, LQ]])
        return eng.dma_start(out=o, in_=i)

    def plan(n_alpha, n_gamma):
        """True = gamma; spread gammas evenly, last instruction alpha"""
        n = n_alpha + n_gamma
        gap = n / n_gamma if n_gamma else 0
        gslots = {int(gap * k) for k in range(n_gamma)}
        sizes = [k in gslots for k in range(n)]
        assert sum(sizes) == n_gamma and not sizes[-1]
        return sizes

    q1, q2 = plan(13, 10), plan(18, 4)
    rows = sum(RGG if g else RGA for g in q1 + q2)
    assert rows == ROWS_PER_CORE, rows

    with nc.Block() as block, nc.semaphore("dma_sem") as dma_sem, \
            nc.semaphore("scr_sem") as scr_sem:

        @block.sync
        def _(sync):
            load(sync).then_inc(dma_sem, 16)
            sync.wait_ge(dma_sem, 16)
            r = 0
            for k, is_g in enumerate(q1):
                ins = store(sync, r, is_g)
                r += RGG if is_g else RGA
                ins.then_inc(dma_sem if k == len(q1) - 1 else scr_sem, 16)
            sync.wait_ge(dma_sem, 48)

        @block.scalar
        def _(scalar):
            scalar.wait_ge(dma_sem, 16)
            r = sum(RGG if g else RGA for g in q1)
            for k, is_g in enumerate(q2):
                ins = store(scalar, r, is_g)
                r += RGG if is_g else RGA
                ins.then_inc(dma_sem if k == len(q2) - 1 else scr_sem, 16)
            scalar.wait_ge(dma_sem, 48)

    return nc


def _pack_value(value):
    vflat = np.ascontiguousarray(np.asarray(value, np.float32)).reshape(ROW_FL)
    wt = np.zeros((NPART, FREE), np.float32)
    for p in range(PA):
        w = vflat[p * SGA: p * SGA + CA]
        for r in range(RGA):
            wt[p, r * CA:(r + 1) * CA] = w
    for p in range(PG):
        w = vflat[p * SGG: p * SGG + CG]
        for r in range(RGG):
            wt[16 + p, r * CG:(r + 1) * CG] = w
    # reorder into load-descriptor order: DRAM chunk k = SBUF quarter
    # (k%4)... desc k of the load writes partition k//4, quarter k%4,
    # and reads DRAM chunk at in-AP position k = packet k//8, desc k%8:
    # in offset = (k//8)*8*LQ + (k%8)*LQ = k*LQ (linear!), so the DRAM
    # buffer is just wt flattened in (partition, quarter) order.
    return wt.reshape(-1)


def kernel(query=None, key=None, value=None, attn_mask=None, **_ignored):
    global LAST_RESULT
    vw = _pack_value(value)

    nc = build_program()
    core_ids = list(range(N_CORES))
    in_maps = [{"value_w": vw} for _ in core_ids]
    res = run_bass_kernel_spmd(nc, in_maps, core_ids, trace=TRACE,
                               **TRACE_KWARGS)
    LAST_RESULT = res

    shards = [res.results[i]["out"].reshape(ROWS_PER_CORE, S, 1, D)
              for i in range(N_CORES)]
    return np.concatenate(shards, axis=0)


# revision 14
# speedup vs baseline: 1.7815x; 1.4510x over previous
"""Trainium2 Bass kernel for nn_Model_39676907882504.

Math: qk = (q @ k^T)/8 has shape [1,2048,1,1]; after the transposes it is
[2048,1,1,1], and softmax over the trailing size-1 axis is exactly 1.0
regardless of qk.  The final matmul with attn_weight == 1 reduces to
broadcasting `value` across a new leading dim:

    output[i, j, 0, :] = value[0, j, 0, :]   for all i in [0, 2048)

i.e. a 512KB -> 1GiB broadcast copy.  Pure memory-regime kernel.
Sharding: 256 output rows per core x 8 cores; value staged in SBUF.

HW model (established by trace analysis + probe kernels this session):
  - A dynamic DMA instruction with a 3-dim DRAM-side AP is split into
    PACKETS, one per outermost-dim index; packet j goes to SDMA engine
    64+(j%16), restarting at 64 every instruction.
  - SBUF AXI port p serves partitions ≡ p (mod 16).  Packets walking
    consecutive partitions make engines drift and collide on ports
    (~58% rate).  Packet j pinned to partitions ≡ j (mod 16) locks
    engine j to port j: no cross-engine collisions.
  - CONSECUTIVE descriptors on one engine from the SAME SBUF partition
    do not pipeline (~5.1us each, vs ~1.25us pipelined) regardless of
    address.  Descriptors from different partitions pipeline, and the
    engine alternates between the two HWDGE queues at ~descriptor
    granularity.  So: queue SP reads partition j, queue ACT reads
    partition 16+j (same port, different partition) - the natural
    queue ping-pong alternates partitions.
  - Engine 79 (index 15) sustains ~21.4 GB/s vs ~26.9 for the rest.

Kernel: an output row is cut into P overlapping windows (window p =
vflat[sigma*p : sigma*p+C]; overlaps rewrite identical bytes, harmless).
One instruction stores RG=8 rows as P packets x 8 descriptors (window j
of 8 consecutive rows, re-reading one partition via a stride-0 dim):

    out AP [[sigma, P], [ROW_FL, 8], [1, C]]
    in  AP [[8740, P], [0, 8], [1, C]]

  alpha: P=16, sigma=8190, C=8222 - all 16 engines, uniform.
  gamma: P=15, sigma=8738, C=8740 - engine 79 idle.

26 alpha + 6 gamma instructions cover 256 rows; engine 79 carries only
alpha work (~320us at 21.4 GB/s), engines 64-78 ~317us: balanced.
SBUF tile [63, 8740]: partitions 0-15 alpha windows for queue SP, 16-31
the same windows for queue ACT, 32-46 / 48-62 likewise for gamma.  Each
queue ends with an alpha instruction whose semaphore inc is the drain
barrier (packet j FIFO-behind everything earlier on engine j).
"""

import sys

for _p in ("/opt/trn_rl_repo",):
    if _p not in sys.path:
        sys.path.insert(0, _p)

import numpy as np

import bass_rust
import concourse.bass as bass
import concourse.mybir as mybir
from concourse.bass_utils import run_bass_kernel_spmd

S = 2048
D = 64
N_CORES = 8
ROWS_PER_CORE = S // N_CORES          # 256
ROW_FL = S * D                        # 131072 floats per output row
RG = 8                                # rows per instruction

PA, SGA, CA = 16, 8190, 8222          # alpha
assert (PA - 1) * SGA + CA == ROW_FL
PG, SGG, CG = 15, 8738, 8740          # gamma (engine 79 idle)
assert (PG - 1) * SGG + CG == ROW_FL

NPART = 63                            # SBUF partitions
PITCH = CG                            # tile free size (8740 floats)
# per queue: 13 alpha + 3 gamma instructions (16 x 8 rows = 128 rows)
GAMMA_SLOTS = {4, 9, 14}

TRACE = False          # test.py flips this to profile
TRACE_KWARGS = {}
LAST_RESULT = None     # BassKernelResults of the last run (for test.py)


def build_program():
    nc = bass.Bass()
    val = nc.declare_dram_parameter("value_w", [NPART, PITCH],
                                    mybir.dt.float32, isOutput=False)
    out = nc.declare_dram_parameter("out", [ROWS_PER_CORE, ROW_FL],
                                    mybir.dt.float32, isOutput=True)
    wt = nc.alloc_sbuf_tensor("wt", [NPART, PITCH], mybir.dt.float32)

    def store(eng, r0, is_gamma, base):
        o = out[r0:r0 + RG, 0:ROW_FL]
        if is_gamma:
            o.ap = bass_rust.VecI64Pair([[SGG, PG], [ROW_FL, RG], [1, CG]])
            i = wt[base:base + PG, 0:CG]
            i.ap = bass_rust.VecI64Pair([[PITCH, PG], [0, RG], [1, CG]])
        else:
            o.ap = bass_rust.VecI64Pair([[SGA, PA], [ROW_FL, RG], [1, CA]])
            i = wt[base:base + PA, 0:CA]
            i.ap = bass_rust.VecI64Pair([[PITCH, PA], [0, RG], [1, CA]])
        return eng.dma_start(out=o, in_=i)

    def plan():
        return [k in GAMMA_SLOTS for k in range(16)]

    half = ROWS_PER_CORE // 2

    with nc.Block() as block, nc.semaphore("dma_sem") as dma_sem, \
            nc.semaphore("scr_sem") as scr_sem:

        @block.sync
        def _(sync):
            sync.dma_start(out=wt[:, :], in_=val[:, :]).then_inc(dma_sem, 16)
            sync.wait_ge(dma_sem, 16)
            for k, is_g in enumerate(plan()):
                ins = store(sync, k * RG, is_g, 32 if is_g else 0)
                ins.then_inc(dma_sem if k == 15 else scr_sem, 16)
            sync.wait_ge(dma_sem, 48)

        @block.scalar
        def _(scalar):
            scalar.wait_ge(dma_sem, 16)
            for k, is_g in enumerate(plan()):
                ins = store(scalar, half + k * RG, is_g, 48 if is_g else 16)
                ins.then_inc(dma_sem if k == 15 else scr_sem, 16)
            scalar.wait_ge(dma_sem, 48)

    return nc


def _pack_value(value):
    vflat = np.ascontiguousarray(np.asarray(value, np.float32)).reshape(ROW_FL)
    wt = np.zeros((NPART, PITCH), np.float32)
    for p in range(PA):
        wt[p, :CA] = vflat[p * SGA: p * SGA + CA]
        wt[16 + p, :CA] = wt[p, :CA]
    for p in range(PG):
        wt[32 + p, :CG] = vflat[p * SGG: p * SGG + CG]
        wt[48 + p, :CG] = wt[32 + p, :CG]
    return wt


def kernel(query=None, key=None, value=None, attn_mask=None, **_ignored):
    global LAST_RESULT
    vw = _pack_value(value)

    nc = build_program()
    core_ids = list(range(N_CORES))
    in_maps = [{"value_w": vw} for _ in core_ids]
    res = run_bass_kernel_spmd(nc, in_maps, core_ids, trace=TRACE,
                               **TRACE_KWARGS)
    LAST_RESULT = res

    shards = [res.results[i]["out"].reshape(ROWS_PER_CORE, S, 1, D)
              for i in range(N_CORES)]
    return np.concatenate(shards, axis=0)


# revision 15
# speedup vs baseline: 3.3468x; 1.8786x over previous
"""Trainium2 Bass kernel for nn_Model_39676907882504.

Math: qk = (q @ k^T)/8 has shape [1,2048,1,1]; after the transposes it is
[2048,1,1,1], and softmax over the trailing size-1 axis is exactly 1.0
regardless of qk.  The final matmul with attn_weight == 1 reduces to
broadcasting `value` across a new leading dim:

    output[i, j, 0, :] = value[0, j, 0, :]   for all i in [0, 2048)

i.e. a 512KB -> 1GiB broadcast copy.  Pure memory-regime kernel.
Sharding: 256 output rows per core x 8 cores; value staged in SBUF.

HW model (established by trace analysis + probe kernels this session):
  - For a 2-dim DRAM-side AP, descriptors are assigned to the 16 SDMA
    engines singly round-robin: desc i -> engine 64+(i%16), restarting
    per instruction.  (3-dim DRAM APs switch to packet-per-outer-index
    assignment whose consecutive-partition runs break port affinity -
    avoid.)
  - SBUF AXI port p serves partitions ≡ p (mod 16), so with the SBUF
    partition dim in descriptor order, engine k only ever touches port
    k: zero port contention (measured 26.9 GB/s/engine = 99% of the
    32B x 850MHz port rate).
  - Pipelining needs same-queue descriptor runs: engines alternate
    between the two HWDGE queues at run boundaries, and each switch
    costs ~2.5-4us unless the run is >=8 descs.  8 descs/engine per
    instruction (128-desc instructions) measured bubble-free.
  - Instructions with 1 desc/engine serialize at ~4.5-5.4us/desc: the
    old per-copy loads burned ~35us; a 64-desc load (4 descs/engine)
    pipelines.
  - SBUF AP partition dim caps descriptors at 128/instruction, and
    desc-count ≢ 0 (mod 16) schemes die on the 128-partition wrap, so
    engine 79's ~21% speed deficit (known quirk) cannot be rebalanced
    in this structure; it sets the tail.

Kernel: SBUF tile [128, 8192]: partition q holds row-chunk (q mod 16) =
vflat[8192*(q%16) : +8192] (host-replicated 8x, uploaded before
execution).  Stores: 16 instructions per queue, each [128, 8192] ->
8 output rows (4 MiB, 8 descs/engine).  Loads: one 64-desc instruction
per queue (partitions 0-63 / 64-127).  Final store per queue doubles as
the drain barrier (per-engine FIFO).
"""

import sys

for _p in ("/opt/trn_rl_repo",):
    if _p not in sys.path:
        sys.path.insert(0, _p)

import numpy as np

import concourse.bass as bass
import concourse.mybir as mybir
from concourse.bass_utils import run_bass_kernel_spmd

S = 2048
D = 64
N_CORES = 8
ROWS_PER_CORE = S // N_CORES          # 256
ROW_FL = S * D                        # 131072 floats per output row
CHUNK = 8192                          # floats per descriptor (32 KiB)
RG = 8                                # rows per store instruction

TRACE = False          # test.py flips this to profile
TRACE_KWARGS = {}
LAST_RESULT = None     # BassKernelResults of the last run (for test.py)


def build_program():
    nc = bass.Bass()
    val = nc.declare_dram_parameter("value_r", [128, CHUNK],
                                    mybir.dt.float32, isOutput=False)
    out = nc.declare_dram_parameter("out", [ROWS_PER_CORE, ROW_FL],
                                    mybir.dt.float32, isOutput=True)
    wt = nc.alloc_sbuf_tensor("wt", [128, CHUNK], mybir.dt.float32)

    def store(eng, r0):
        return eng.dma_start(
            out=out[r0:r0 + RG, 0:ROW_FL].rearrange(
                "r (p c) -> (r p) c", p=16),
            in_=wt[0:128, 0:CHUNK])

    half = ROWS_PER_CORE // 2
    n_ins = half // RG                                # 16 per queue

    with nc.Block() as block, nc.semaphore("dma_sem") as dma_sem, \
            nc.semaphore("scr_sem") as scr_sem:

        @block.sync
        def _(sync):
            sync.dma_start(out=wt[0:64, :],
                           in_=val[0:64, :]).then_inc(dma_sem, 16)
            sync.wait_ge(dma_sem, 32)
            for k in range(n_ins):
                ins = store(sync, k * RG)
                ins.then_inc(dma_sem if k == n_ins - 1 else scr_sem, 16)
            sync.wait_ge(dma_sem, 64)

        @block.scalar
        def _(scalar):
            scalar.dma_start(out=wt[64:128, :],
                             in_=val[64:128, :]).then_inc(dma_sem, 16)
            scalar.wait_ge(dma_sem, 32)
            for k in range(n_ins):
                ins = store(scalar, half + k * RG)
                ins.then_inc(dma_sem if k == n_ins - 1 else scr_sem, 16)
            scalar.wait_ge(dma_sem, 64)

    return nc


def kernel(query=None, key=None, value=None, attn_mask=None, **_ignored):
    global LAST_RESULT
    vflat = np.ascontiguousarray(np.asarray(value, np.float32)).reshape(ROW_FL)
    vr = np.tile(vflat.reshape(16, CHUNK), (8, 1))

    nc = build_program()
    core_ids = list(range(N_CORES))
    in_maps = [{"value_r": vr} for _ in core_ids]
    res = run_bass_kernel_spmd(nc, in_maps, core_ids, trace=TRACE,
                               **TRACE_KWARGS)
    LAST_RESULT = res

    shards = [res.results[i]["out"].reshape(ROWS_PER_CORE, S, 1, D)
              for i in range(N_CORES)]
    return np.concatenate(shards, axis=0)
